# revision 1
# baseline (speedup 1.0000x reference)
"""Trainium2 Bass kernel for nn_CEA_10247791968685 (sparse_attention).

Reference computation (per batch):
  qk = GLU(Conv1d(x, w_conv, k=3, pad=1))        # [n, 512]
  q = split_heads(qk @ wq); k = split_heads(qk @ wk); v = split_heads(x @ wv)
  k_sm = softmax(k, axis=n); ctx = einsum("nd,ne->de", k_sm, v) * d**-0.5
  q_sm = softmax(q, axis=d); out = einsum("nd,de->ne", q_sm, ctx)
  return merge_heads(out) @ wo + bo

Sharding: 8 cores = 4 batches x 2 head-groups (4 heads each).  Each core
computes the full conv+GLU for its batch and the attention for its 4 heads;
the final projection uses the head-group's rows of wo, so the two cores of a
batch produce partial sums that the host adds (plus bias).

All heavy matmuls run in float32r (fp32 bits, PE rounds internally; 4x faster
than fp32 at free-dim>=256, ~1.5e-4 relative error).  Softmaxes use
unstabilized exp (values are O(3), fp32-safe) so no cross-n max is needed.
The per-(h,d) softmax-denominator for k is accumulated as an extra PSUM
column of the context-matrix accumulation (rhs = ones).  GLU uses the native
Sigmoid ACT op: measured faster on HW than an exp-table-only chain, despite
the act-table reloads the cost model charges for it.
"""

import os
import sys

for _p in ("/opt/trn_rl_repo", "/root/.axon_site/_ro/trn_rl_repo"):
    if os.path.isdir(_p) and _p not in sys.path:
        sys.path.insert(0, _p)

import numpy as np

B = 4
N = 4096
DIM = 512
HEADS = 8
DH = 64
INNER = HEADS * DH  # 512
SCALE = DH**-0.5  # 0.125
G = 2  # head groups (cores per batch)
GH = HEADS // G  # heads per core = 4
EG = GH * DH  # per-core inner width = 256
NB = 512  # streamed sequence block
NBLK = N // NB  # 8
NT = N // 128  # 32 n-tiles
CK = DIM // 128  # 4 contraction tiles over channels


def build_kernel(repeat: int = 1, loop_n: int = 1, glu_sigmoid: bool = True):
    import concourse.mybir as mybir
    import concourse.tile as tile
    from concourse import bacc
    from concourse.masks import make_identity

    f32 = mybir.dt.float32
    f32r = mybir.dt.float32r
    AF = mybir.ActivationFunctionType
    mult = mybir.AluOpType.mult
    AX = mybir.AxisListType.X

    nc = bacc.Bacc("TRN2", target_bir_lowering=False, debug=False)

    # Per-core inputs (host pre-sharded / pre-transposed).
    xT = nc.dram_tensor("xT", [DIM, N + 2], f32r, kind="ExternalInput")
    wct = nc.dram_tensor("wct", [3, DIM, 2 * DIM], f32r, kind="ExternalInput")
    wq = nc.dram_tensor("wq", [DIM, EG], f32r, kind="ExternalInput")
    wk = nc.dram_tensor("wk", [DIM, EG], f32r, kind="ExternalInput")
    wv = nc.dram_tensor("wv", [DIM, EG], f32r, kind="ExternalInput")
    wo = nc.dram_tensor("wo", [EG, DIM], f32r, kind="ExternalInput")
    out = nc.dram_tensor("out", [N, DIM], f32, kind="ExternalOutput")

    with tile.TileContext(nc) as tc:
        with (
            tc.tile_pool(name="const", bufs=1) as p_const,
            tc.tile_pool(name="wts", bufs=1) as p_wts,
        ):
            ident = p_const.tile([128, 128], f32)
            make_identity(nc, ident)
            # memset cannot produce float32r (walrus FP32r-producer rule);
            # stage in f32 and copy (DVE copy is an accepted producer).
            ones_f = p_const.tile([128, 2], f32)
            nc.vector.memset(ones_f[:], 1.0)
            ones = p_const.tile([128, 2], f32r)
            nc.vector.tensor_copy(ones[:], ones_f[:])
            zeros_f = p_const.tile([128, 128], f32)
            nc.vector.memset(zeros_f[:], 0.0)

            # Projection weights, resident: [128, CK, EG] (K-major blocks).
            # (DMAs are issued inside _body, ordered by first use.)
            wq_sb = p_wts.tile([128, CK, EG], f32r, tag="wq")
            wk_sb = p_wts.tile([128, CK, EG], f32r, tag="wk")
            wv_sb = p_wts.tile([128, CK, EG], f32r, tag="wv")
            wo_sb = p_wts.tile([128, EG // 128, DIM], f32r, tag="wo")

            if loop_n > 1:
                with tc.For_i(0, loop_n, 1):
                    _body(nc, tc, mybir, xT, wct, out, ident, ones, zeros_f,
                          wq_sb, wk_sb, wv_sb, wo_sb, wq, wk, wv, wo,
                          glu_sigmoid)
            else:
                for _rep in range(repeat):
                    _body(nc, tc, mybir, xT, wct, out, ident, ones, zeros_f,
                          wq_sb, wk_sb, wv_sb, wo_sb, wq, wk, wv, wo,
                          glu_sigmoid)

    nc.compile()
    return nc


def _body(nc, tc, mybir, xT, wct, out, ident, ones, zeros_f, wq_sb, wk_sb, wv_sb, wo_sb, wq, wk, wv, wo, glu_sigmoid=False):
    f32 = mybir.dt.float32
    f32r = mybir.dt.float32r
    AF = mybir.ActivationFunctionType
    mult = mybir.AluOpType.mult
    AX = mybir.AxisListType.X
    NPAIR = GH // 2  # head pairs per core = 2

    with (
        tc.tile_pool(name="conv_w", bufs=1) as p_cw,
        tc.tile_pool(name="stream", bufs=3) as p_st,
        tc.tile_pool(name="glu", bufs=2) as p_glu,
        tc.tile_pool(name="acts", bufs=1) as p_acts,
        tc.tile_pool(name="sm", bufs=3) as p_sm,
        tc.tile_pool(name="ps_es", bufs=1, space="PSUM") as ps_es,
    ):
        # Conv weights resident, as 32 per-(ck, o-block) tiles loaded in the
        # order the conv consumes them (ct=0's A+G blocks first) so the first
        # matmuls don't wait for the whole 6.3 MB of w_conv.
        wc_sb = [
            [
                p_cw.tile([128, 3, 128], f32r, tag=f"wc{ck}o{ot}", name=f"wc{ck}o{ot}")
                for ot in range(2 * CK)
            ]
            for ck in range(CK)
        ]

        # First x block before everything (the first conv matmuls need it).
        xs0 = p_st.tile([128, CK, NB + 2], f32r, tag="xs", name="xs0")
        for ck in range(CK):
            nc.sync.dma_start(
                xs0[:, ck, :], xT.ap()[ck * 128 : (ck + 1) * 128, 0 : NB + 2]
            )

        def load_wc(ct):
            for ot in (ct, CK + ct):
                for ck in range(CK):
                    nc.sync.dma_start(
                        wc_sb[ck][ot][:],
                        wct.ap()[
                            :, ck * 128 : (ck + 1) * 128, ot * 128 : (ot + 1) * 128
                        ].rearrange("t c o -> c t o"),
                    )

        load_wc(0)
        for t_sb, t_dr in ((wk_sb, wk), (wq_sb, wq)):
            nc.sync.dma_start(
                t_sb[:], t_dr.ap().rearrange("(ck p) e -> p ck e", p=128)
            )
        load_wc(1)
        nc.sync.dma_start(wv_sb[:], wv.ap().rearrange("(ck p) e -> p ck e", p=128))
        load_wc(2)
        load_wc(3)
        nc.sync.dma_start(wo_sb[:], wo.ap().rearrange("(ep p) f -> p ep f", p=128))

        # Persistent outputs of the stream phase.
        qsmT = p_acts.tile([128, NPAIR, N], f32r, tag="qsmT")  # [e(pair), n]
        # Pinned PSUM accumulators: per pair [128, 257]: cols 0:256 = E over
        # all 4 local heads' v columns, col 256 = sum(exp_k) (rhs=ones).
        psES = [ps_es.tile([128, 512], f32, tag=f"es{p}", name=f"es{p}") for p in range(NPAIR)]

        with (
            tc.tile_pool(name="ps_convA", bufs=2, space="PSUM") as ps_convA,
            tc.tile_pool(name="ps_convG", bufs=1, space="PSUM") as ps_convG,
            tc.tile_pool(name="ps_work", bufs=3, space="PSUM") as ps_work,
        ):
          for nb in range(NBLK):
            # ---- load x block (with conv halo; nb=0 prefetched above) ----
            if nb == 0:
                xs = xs0
            else:
                xs = p_st.tile([128, CK, NB + 2], f32r, tag="xs")
                for ck in range(CK):
                    nc.sync.dma_start(
                        xs[:, ck, :],
                        xT.ap()[
                            ck * 128 : (ck + 1) * 128, nb * NB : nb * NB + NB + 2
                        ],
                    )

            # ---- conv + GLU -> qk block [128, CK, NB] (channel-major) ----
            qk = p_st.tile([128, CK, NB], f32r, tag="qk")
            for ct in range(CK):
                psA = ps_convA.tile([128, NB], f32, tag="pA")
                psG = ps_convG.tile([128, NB], f32, tag="pG")
                n_mm = 3 * CK
                i = 0
                for t in range(3):
                    for ck in range(CK):
                        rhs = xs[:, ck, t : t + NB]
                        nc.tensor.matmul(
                            psA[:],
                            wc_sb[ck][ct][:, t, :],
                            rhs,
                            start=(i == 0),
                            stop=(i == n_mm - 1),
                        )
                        nc.tensor.matmul(
                            psG[:],
                            wc_sb[ck][CK + ct][:, t, :],
                            rhs,
                            start=(i == 0),
                            stop=(i == n_mm - 1),
                        )
                        i += 1
                if glu_sigmoid:
                    sig = p_glu.tile([128, NB], f32, tag="sig")
                    nc.scalar.activation(sig[:], psG[:], AF.Sigmoid)
                    nc.vector.tensor_tensor(qk[:, ct, :], psA[:], sig[:], mult)
                else:
                    # GLU via the exp table only (a Sigmoid ACT op would force
                    # an act-table reload between every sigmoid and exp):
                    #   qk = psA / (1 + exp(-g))
                    sig = p_glu.tile([128, NB], f32, tag="sig")
                    nc.scalar.activation(sig[:], psG[:], AF.Exp, scale=-1.0)
                    nc.vector.tensor_scalar_add(sig[:], sig[:], 1.0)
                    rcp = p_glu.tile([128, NB], f32, tag="rcp")
                    nc.vector.reciprocal(rcp[:], sig[:])
                    nc.vector.tensor_tensor(qk[:, ct, :], psA[:], rcp[:], mult)

            # ---- projections + softmaxes for the 4 n-tiles of this block ----
            for lnt in range(NB // 128):
                nt = nb * (NB // 128) + lnt
                nsl = slice(lnt * 128, lnt * 128 + 128)

                # k -> exp(k)
                psk = ps_work.tile([128, EG], f32, tag="work")
                for ck in range(CK):
                    nc.tensor.matmul(
                        psk[:], qk[:, ck, nsl], wk_sb[:, ck, :],
                        start=(ck == 0), stop=(ck == CK - 1),
                    )
                ek = p_sm.tile([128, EG], f32r, tag="ek")
                nc.scalar.activation(ek[:], psk[:], AF.Exp)

                # v
                psv = ps_work.tile([128, EG], f32, tag="work")
                for ck in range(CK):
                    nc.tensor.matmul(
                        psv[:], xs[:, ck, 1 + lnt * 128 : 1 + lnt * 128 + 128],
                        wv_sb[:, ck, :],
                        start=(ck == 0), stop=(ck == CK - 1),
                    )
                # [v | ones]: the ones column makes the E matmul also
                # accumulate S = sum_n exp_k (saves a separate S matmul and
                # its redundant LDWEIGHTS of the same ek chunk; N=258 stays
                # in the fast fp32r regime).
                va = p_sm.tile([128, EG + 2], f32r, tag="vt")
                nc.scalar.copy(va[:, 0:EG], psv[:])
                nc.vector.tensor_copy(va[:, EG : EG + 2], ones[:])

                for p in range(NPAIR):
                    nc.tensor.matmul(
                        psES[p][:, 0 : EG + 2],
                        ek[:, p * 128 : p * 128 + 128],
                        va[:],
                        start=(nt == 0), stop=(nt == NT - 1),
                    )

                # q -> softmax over d (free sub-ranges of 64) -> transpose
                psq = ps_work.tile([128, EG], f32, tag="work")
                for ck in range(CK):
                    nc.tensor.matmul(
                        psq[:], qk[:, ck, nsl], wq_sb[:, ck, :],
                        start=(ck == 0), stop=(ck == CK - 1),
                    )
                eq = p_sm.tile([128, EG], f32, tag="eq")
                nc.scalar.activation(eq[:], psq[:], AF.Exp)
                eq3 = eq[:].rearrange("p (h d) -> p h d", d=DH)
                s4 = p_sm.tile([128, GH, 1], f32, tag="s4")
                nc.vector.reduce_sum(s4[:], eq3, axis=AX)
                r4 = p_sm.tile([128, GH, 1], f32, tag="r4")
                nc.vector.reciprocal(r4[:], s4[:])
                qsm = p_sm.tile([128, EG], f32, tag="qsm")
                nc.vector.tensor_tensor(
                    qsm[:].rearrange("p (h d) -> p h d", d=DH),
                    eq3,
                    r4[:].to_broadcast((128, GH, DH)),
                    mult,
                )
                for eb in range(NPAIR):
                    psT = ps_work.tile([128, 128], f32, tag="work")
                    nc.tensor.transpose(
                        psT[:], qsm[:, eb * 128 : eb * 128 + 128], ident[:]
                    )
                    nc.vector.tensor_copy(qsmT[:, eb, nt * 128 : nt * 128 + 128], psT[:])

        # ---- block-diagonal ctx assembly:  bd[p] = blockdiag(E_h * SCALE/S) ----
        with (
            tc.tile_pool(name="fin", bufs=3) as p_fin,
            tc.tile_pool(name="bd", bufs=1) as p_bd,
            tc.tile_pool(name="ps_fin", bufs=3, space="PSUM") as ps_fin,
        ):
            bds = []
            for p in range(NPAIR):
                rs = p_bd.tile([128, 1], f32, tag=f"rs{p}")
                nc.vector.reciprocal(rs[:], psES[p][:, EG : EG + 1])
                nc.vector.tensor_scalar_mul(rs[:], rs[:], float(SCALE))
                bd = p_bd.tile([128, 128], f32r, tag=f"bd{p}")
                nc.vector.tensor_copy(bd[:], zeros_f[:])
                for hh in range(2):
                    h_local = 2 * p + hh
                    sl_p = slice(hh * DH, hh * DH + DH)
                    nc.vector.tensor_scalar(
                        bd[sl_p, sl_p],
                        psES[p][sl_p, h_local * DH : h_local * DH + DH],
                        rs[sl_p],
                        None,
                        op0=mult,
                    )
                bds.append(bd)

            # outT[e', n] = bd[p].T @ qsmT[p], then immediately the final
            # projection for that 512-block; output DMAs batched 4 n-tiles.
            outT = p_bd.tile([128, NPAIR, NB], f32r, tag="outT")
            for nbk in range(NBLK):
                outT = p_fin.tile([128, NPAIR, NB], f32r, tag="outT")
                for p in range(NPAIR):
                    pso = ps_fin.tile([128, NB], f32, tag="pso")
                    nc.tensor.matmul(
                        pso[:], bds[p][:], qsmT[:, p, nbk * NB : nbk * NB + NB],
                        start=True, stop=True,
                    )
                    nc.scalar.copy(outT[:, p, :], pso[:])
                fo = p_fin.tile([128, NB // 128, DIM], f32, tag="fo")
                for lnt in range(NB // 128):
                    psf = ps_fin.tile([128, DIM], f32, tag="psf")
                    for ep in range(EG // 128):
                        nc.tensor.matmul(
                            psf[:], outT[:, ep, lnt * 128 : lnt * 128 + 128],
                            wo_sb[:, ep, :],
                            start=(ep == 0), stop=(ep == EG // 128 - 1),
                        )
                    nc.vector.tensor_copy(fo[:, lnt, :], psf[:])
                nc.sync.dma_start(
                    out.ap()[nbk * NB : (nbk + 1) * NB, :].rearrange(
                        "(i p) f -> p i f", p=128
                    ),
                    fo[:],
                )


# ---------------------------------------------------------------------------
# Host-side: shard, run on 8 cores via PJRT (axon), gather.
# ---------------------------------------------------------------------------


class _Runner:
    """Compile once, execute many times (run_bass_kernel_spmd re-jits per call)."""

    def __init__(self, nc, n_cores: int):
        import jax
        from jax.sharding import Mesh, PartitionSpec
        from jax.experimental.shard_map import shard_map
        import concourse.mybir as mybir
        from concourse.bass2jax import (
            _bass_exec_p,
            install_neuronx_cc_hook,
            partition_id_tensor,
        )

        install_neuronx_cc_hook()
        self.jax = jax
        self.n_cores = n_cores
        partition_name = nc.partition_id_tensor.name if nc.partition_id_tensor else None
        in_names, out_names, out_avals, zero_outs = [], [], [], []
        for alloc in nc.m.functions[0].allocations:
            if not isinstance(alloc, mybir.MemoryLocationSet):
                continue
            name = alloc.memorylocations[0].name
            if alloc.kind == "ExternalInput":
                if name != partition_name:
                    in_names.append(name)
            elif alloc.kind == "ExternalOutput":
                shape = tuple(alloc.tensor_shape)
                dtype = mybir.dt.np(alloc.dtype)
                out_names.append(name)
                out_avals.append(jax.core.ShapedArray(shape, dtype))
                zero_outs.append(np.zeros(shape, dtype))
        self.in_names = in_names
        self.out_names = out_names
        self.out_avals = out_avals
        self.zero_outs = zero_outs
        n_params = len(in_names)
        self.n_params = n_params
        all_in_names = in_names + out_names
        if partition_name is not None:
            all_in_names.append(partition_name)
        donate = tuple(range(n_params, n_params + len(out_avals)))

        def _pbody(*args):
            operands = list(args)
            if partition_name is not None:
                operands.append(partition_id_tensor())
            outs = _bass_exec_p.bind(
                *operands,
                out_avals=tuple(out_avals),
                in_names=tuple(all_in_names),
                out_names=tuple(out_names),
                lowering_input_output_aliases=(),
                sim_require_finite=True,
                sim_require_nnan=True,
                nc=nc,
            )
            return tuple(outs)

        devices = jax.devices()[:n_cores]
        mesh = Mesh(np.asarray(devices), ("core",))
        in_specs = (PartitionSpec("core"),) * (n_params + len(out_avals))
        out_specs = (PartitionSpec("core"),) * len(out_names)
        self._fn = jax.jit(
            shard_map(
                _pbody, mesh=mesh, in_specs=in_specs, out_specs=out_specs,
                check_rep=False,
            ),
            donate_argnums=donate,
            keep_unused=True,
        )

    def __call__(self, in_maps):
        n_cores = self.n_cores
        per_core = [[np.asarray(m[n]) for n in self.in_names] for m in in_maps]
        concat_in = [
            np.concatenate([per_core[c][i] for c in range(n_cores)], axis=0)
            for i in range(self.n_params)
        ]
        concat_zeros = [
            np.zeros((n_cores * z.shape[0], *z.shape[1:]), z.dtype)
            for z in self.zero_outs
        ]
        out_arrs = self._fn(*concat_in, *concat_zeros)
        self.jax.block_until_ready(out_arrs)
        return [
            {
                n: np.asarray(out_arrs[i]).reshape(
                    n_cores, *self.out_avals[i].shape
                )[c]
                for i, n in enumerate(self.out_names)
            }
            for c in range(n_cores)
        ]


_CACHE = {}


def _get_runner(repeat: int = 1):
    key = repeat
    if key not in _CACHE:
        nc = build_kernel(repeat)
        _CACHE[key] = _Runner(nc, 8)
    return _CACHE[key]


def make_in_maps(x, w_conv, wq, wk, wv, wo):
    """Host-side shard: returns the 8 per-core input dicts."""
    x = np.asarray(x, np.float32)
    xTp = np.zeros((B, DIM, N + 2), np.float32)
    xTp[:, :, 1 : N + 1] = x.transpose(0, 2, 1)
    wct = np.ascontiguousarray(np.asarray(w_conv, np.float32).transpose(2, 1, 0))
    wq = np.asarray(wq, np.float32)
    wk = np.asarray(wk, np.float32)
    wv = np.asarray(wv, np.float32)
    wo = np.asarray(wo, np.float32)
    in_maps = []
    for c in range(8):
        b, g = c // G, c % G
        sl = slice(g * EG, (g + 1) * EG)
        in_maps.append(
            {
                "xT": np.ascontiguousarray(xTp[b]),
                "wct": wct,
                "wq": np.ascontiguousarray(wq[:, sl]),
                "wk": np.ascontiguousarray(wk[:, sl]),
                "wv": np.ascontiguousarray(wv[:, sl]),
                "wo": np.ascontiguousarray(wo[sl, :]),
            }
        )
    return in_maps


def kernel(x, w_conv, wq, wk, wv, wo, bo):
    runner = _get_runner(1)
    in_maps = make_in_maps(x, w_conv, wq, wk, wv, wo)
    res = runner(in_maps)
    bo = np.asarray(bo, np.float32)
    out = np.empty((B, N, DIM), np.float32)
    for b in range(B):
        out[b] = res[G * b]["out"] + res[G * b + 1]["out"] + bo
    return out


# ---------------------------------------------------------------------------
# v2: batch x seq-half sharding.  Each core does conv+attention for half the
# sequence of one batch with ALL 8 heads; the k-softmax denominator and the
# d x d context matrices are globally summed over the sequence with a paired
# AllReduce ([[0,1],[2,3],[4,5],[6,7]]).  Conv work per core halves vs v1.
# ---------------------------------------------------------------------------

NH = N // 2  # per-core sequence half = 2048
NBLK2 = NH // NB  # 4
NT2 = NH // 128  # 16
NP2 = HEADS // 2  # 4 head pairs
RG = [[0, 1], [2, 3], [4, 5], [6, 7]]


def build_kernel_v2(repeat: int = 1, loop_n: int = 1, no_cc: bool = False):
    import concourse.mybir as mybir
    import concourse.tile as tile
    from concourse import bacc
    from concourse.masks import make_identity

    f32 = mybir.dt.float32
    f32r = mybir.dt.float32r

    nc = bacc.Bacc("TRN2", target_bir_lowering=False, debug=False, num_devices=8)

    xT = nc.dram_tensor("xT", [DIM, NH + 2], f32r, kind="ExternalInput")
    wct = nc.dram_tensor("wct", [3, DIM, 2 * DIM], f32r, kind="ExternalInput")
    wq = nc.dram_tensor("wq", [DIM, INNER], f32r, kind="ExternalInput")
    wk = nc.dram_tensor("wk", [DIM, INNER], f32r, kind="ExternalInput")
    wv = nc.dram_tensor("wv", [DIM, INNER], f32r, kind="ExternalInput")
    wo = nc.dram_tensor("wo", [INNER, DIM], f32r, kind="ExternalInput")
    out = nc.dram_tensor("out", [NH, DIM], f32, kind="ExternalOutput")
    cc_in = nc.dram_tensor("cc_in", [(NP2 // 2) * 128, 260], f32, kind="Internal")
    cc_out = nc.dram_tensor("cc_out", [(NP2 // 2) * 128, 260], f32, kind="Internal")

    with tile.TileContext(nc) as tc:
        with (
            tc.tile_pool(name="const", bufs=1) as p_const,
            tc.tile_pool(name="wts", bufs=1) as p_wts,
            tc.tile_pool(name="acts", bufs=1) as p_acts,
        ):
            ident = p_const.tile([128, 128], f32)
            make_identity(nc, ident)
            ones_f = p_const.tile([128, 2], f32)
            nc.vector.memset(ones_f[:], 1.0)
            ones = p_const.tile([128, 2], f32r)
            nc.vector.tensor_copy(ones[:], ones_f[:])
            zeros_f = p_const.tile([128, 128], f32)
            nc.vector.memset(zeros_f[:], 0.0)

            wq_sb = p_wts.tile([128, CK, INNER], f32r, tag="wq")
            wk_sb = p_wts.tile([128, CK, INNER], f32r, tag="wk")
            wv_sb = p_wts.tile([128, CK, INNER], f32r, tag="wv")
            wo_sb = p_wts.tile([128, INNER // 128, DIM], f32r, tag="wo")

            args = (nc, tc, mybir, xT, wct, out, cc_in, cc_out, ident, ones,
                    zeros_f, wq_sb, wk_sb, wv_sb, wo_sb, wq, wk, wv, wo, p_acts,
                    no_cc)
            if loop_n > 1:
                with tc.For_i(0, loop_n, 1):
                    _body2(*args)
            else:
                for _rep in range(repeat):
                    _body2(*args)

    nc.compile()
    return nc


def _body2(nc, tc, mybir, xT, wct, out, cc_in, cc_out, ident, ones, zeros_f,
           wq_sb, wk_sb, wv_sb, wo_sb, wq, wk, wv, wo, p_acts, no_cc=False):
    f32 = mybir.dt.float32
    f32r = mybir.dt.float32r
    AF = mybir.ActivationFunctionType
    mult = mybir.AluOpType.mult
    AX = mybir.AxisListType.X

    qsmT = p_acts.tile([128, NP2, NH], f32r, tag="qsmT")
    es_loc = p_acts.tile([128, NP2 // 2, 260], f32, tag="es_loc")
    esr = p_acts.tile([128, NP2 // 2, 260], f32, tag="esr")

    with (
        tc.tile_pool(name="conv_w", bufs=1) as p_cw,
        tc.tile_pool(name="stream", bufs=3) as p_st,
        tc.tile_pool(name="glu", bufs=2) as p_glu,
        tc.tile_pool(name="sm", bufs=2) as p_sm,
    ):
        wc_sb = [
            [
                p_cw.tile([128, 3, 128], f32r, tag=f"wc{ck}o{ot}", name=f"wc{ck}o{ot}")
                for ot in range(2 * CK)
            ]
            for ck in range(CK)
        ]

        xs0 = p_st.tile([128, CK, NB + 2], f32r, tag="xs", name="xs0")
        for ck in range(CK):
            nc.sync.dma_start(
                xs0[:, ck, :], xT.ap()[ck * 128 : (ck + 1) * 128, 0 : NB + 2]
            )

        def load_wc(ct):
            for ot in (ct, CK + ct):
                for ck in range(CK):
                    nc.sync.dma_start(
                        wc_sb[ck][ot][:],
                        wct.ap()[
                            :, ck * 128 : (ck + 1) * 128, ot * 128 : (ot + 1) * 128
                        ].rearrange("t c o -> c t o"),
                    )

        load_wc(0)
        for t_sb, t_dr in ((wk_sb, wk), (wq_sb, wq)):
            nc.sync.dma_start(
                t_sb[:], t_dr.ap().rearrange("(ck p) e -> p ck e", p=128)
            )
        load_wc(1)
        nc.sync.dma_start(wv_sb[:], wv.ap().rearrange("(ck p) e -> p ck e", p=128))
        load_wc(2)
        load_wc(3)
        nc.sync.dma_start(wo_sb[:], wo.ap().rearrange("(ep p) f -> p ep f", p=128))

        with (
            tc.tile_pool(name="ps_conv", bufs=2, space="PSUM") as ps_conv,
            tc.tile_pool(name="ps_work", bufs=2, space="PSUM") as ps_work,
            tc.tile_pool(name="ps_es", bufs=1, space="PSUM") as ps_es,
        ):
            # Two head-pairs share one accumulator bank: pair p lives at
            # columns (p%2)*130 .. +130 of bank p//2 ([v(2 heads)|S|S]).
            psES = [
                ps_es.tile([128, 260], f32, tag=f"esb{tp}", name=f"esb{tp}")
                for tp in range(NP2 // 2)
            ]

            for nb in range(NBLK2):
                if nb == 0:
                    xs = xs0
                else:
                    xs = p_st.tile([128, CK, NB + 2], f32r, tag="xs")
                    for ck in range(CK):
                        nc.sync.dma_start(
                            xs[:, ck, :],
                            xT.ap()[
                                ck * 128 : (ck + 1) * 128, nb * NB : nb * NB + NB + 2
                            ],
                        )

                qk = p_st.tile([128, CK, NB], f32r, tag="qk")
                for ct in range(CK):
                    psA = ps_conv.tile([128, NB], f32, tag="pA")
                    psG = ps_conv.tile([128, NB], f32, tag="pG")
                    n_mm = 3 * CK
                    i = 0
                    for t in range(3):
                        for ck in range(CK):
                            rhs = xs[:, ck, t : t + NB]
                            nc.tensor.matmul(
                                psA[:], wc_sb[ck][ct][:, t, :], rhs,
                                start=(i == 0), stop=(i == n_mm - 1),
                            )
                            nc.tensor.matmul(
                                psG[:], wc_sb[ck][CK + ct][:, t, :], rhs,
                                start=(i == 0), stop=(i == n_mm - 1),
                            )
                            i += 1
                    sig = p_glu.tile([128, NB], f32, tag="sig")
                    nc.scalar.activation(sig[:], psG[:], AF.Sigmoid)
                    nc.vector.tensor_tensor(qk[:, ct, :], psA[:], sig[:], mult)

                for lnt in range(NB // 128):
                    nt = nb * (NB // 128) + lnt
                    nsl = slice(lnt * 128, lnt * 128 + 128)

                    # k -> exp(k)
                    psk = ps_work.tile([128, INNER], f32, tag="work")
                    for ck in range(CK):
                        nc.tensor.matmul(
                            psk[:], qk[:, ck, nsl], wk_sb[:, ck, :],
                            start=(ck == 0), stop=(ck == CK - 1),
                        )
                    ek = p_sm.tile([128, INNER], f32r, tag="ek")
                    nc.scalar.activation(ek[:], psk[:], AF.Exp)

                    # v -> [v | ones] per 4-head group
                    psv = ps_work.tile([128, INNER], f32, tag="work")
                    for ck in range(CK):
                        nc.tensor.matmul(
                            psv[:], xs[:, ck, 1 + lnt * 128 : 1 + lnt * 128 + 128],
                            wv_sb[:, ck, :],
                            start=(ck == 0), stop=(ck == CK - 1),
                        )
                    va = p_sm.tile([128, NP2, 130], f32r, tag="va")
                    for p in range(NP2):
                        nc.scalar.copy(
                            va[:, p, 0:128], psv[:, p * 128 : p * 128 + 128]
                        )
                        nc.vector.tensor_copy(va[:, p, 128:130], ones[:])

                    # E/S accumulation; bank p//2 is started by its first
                    # touching matmul (pair 2*(p//2)) and stopped by its last.
                    first = nt == 0
                    last = nt == NT2 - 1
                    for p in range(NP2):
                        nc.tensor.matmul(
                            psES[p // 2][:, (p % 2) * 130 : (p % 2) * 130 + 130],
                            ek[:, p * 128 : p * 128 + 128],
                            va[:, p, :],
                            start=(first and p % 2 == 0),
                            stop=(last and p % 2 == 1),
                        )

                    # q -> softmax over d -> transpose
                    psq = ps_work.tile([128, INNER], f32, tag="work")
                    for ck in range(CK):
                        nc.tensor.matmul(
                            psq[:], qk[:, ck, nsl], wq_sb[:, ck, :],
                            start=(ck == 0), stop=(ck == CK - 1),
                        )
                    eq = p_sm.tile([128, INNER], f32, tag="eq")
                    nc.scalar.activation(eq[:], psq[:], AF.Exp)
                    eq3 = eq[:].rearrange("p (h d) -> p h d", d=DH)
                    s8 = p_sm.tile([128, HEADS, 1], f32, tag="s8")
                    nc.vector.reduce_sum(s8[:], eq3, axis=AX)
                    r8 = p_sm.tile([128, HEADS, 1], f32, tag="r8")
                    nc.vector.reciprocal(r8[:], s8[:])
                    qsm = p_sm.tile([128, INNER], f32, tag="qsm")
                    nc.vector.tensor_tensor(
                        qsm[:].rearrange("p (h d) -> p h d", d=DH),
                        eq3,
                        r8[:].to_broadcast((128, HEADS, DH)),
                        mult,
                    )
                    for eb in range(NP2):
                        psT = ps_work.tile([128, 128], f32, tag="work")
                        nc.tensor.transpose(
                            psT[:], qsm[:, eb * 128 : eb * 128 + 128], ident[:]
                        )
                        nc.vector.tensor_copy(
                            qsmT[:, eb, nt * 128 : nt * 128 + 128], psT[:]
                        )

            # evacuate the pinned E/S accumulators so the PSUM pool can close
            for tp in range(NP2 // 2):
                nc.vector.tensor_copy(es_loc[:, tp, :], psES[tp][:])

    # ---- paired AllReduce of E/S ----
    if no_cc:
        # timing-only variant (collectives cannot re-execute inside For_i):
        # same DRAM round-trip, no wire exchange.
        nc.sync.dma_start(
            cc_in.ap().rearrange("(g p) e -> p g e", p=128), es_loc[:]
        )
        nc.sync.dma_start(
            esr[:], cc_in.ap().rearrange("(g p) e -> p g e", p=128)
        )
    else:
        nc.sync.dma_start(
            cc_in.ap().rearrange("(g p) e -> p g e", p=128), es_loc[:]
        )
        nc.gpsimd.collective_compute(
            "AllReduce",
            mybir.AluOpType.add,
            replica_groups=RG,
            ins=[cc_in.ap()],
            outs=[cc_out.ap()],
        )
        nc.sync.dma_start(
            esr[:], cc_out.ap().rearrange("(g p) e -> p g e", p=128)
        )

    # ---- blockdiag ctx, outT, final projection ----
    with (
        tc.tile_pool(name="fin", bufs=2) as p_fin,
        tc.tile_pool(name="bd", bufs=1) as p_bd,
        tc.tile_pool(name="ps_fin", bufs=3, space="PSUM") as ps_fin,
    ):
        bds = []
        for p in range(NP2):
            tp, off = p // 2, (p % 2) * 130
            rs = p_bd.tile([128, 1], f32, tag=f"rs{p}", name=f"rs{p}")
            nc.vector.reciprocal(rs[:], esr[:, tp, off + 128 : off + 129])
            nc.vector.tensor_scalar_mul(rs[:], rs[:], float(SCALE))
            bd = p_bd.tile([128, 128], f32r, tag=f"bd{p}", name=f"bd{p}")
            nc.vector.tensor_copy(bd[:], zeros_f[:])
            for hh in range(2):
                sl_p = slice(hh * DH, hh * DH + DH)
                col = off + hh * DH
                nc.vector.tensor_scalar(
                    bd[sl_p, sl_p],
                    esr[sl_p, tp, col : col + DH],
                    rs[sl_p],
                    None,
                    op0=mult,
                )
            bds.append(bd)

        for nbk in range(NBLK2):
            outT = p_fin.tile([128, NP2, NB], f32r, tag="outT")
            for p in range(NP2):
                pso = ps_fin.tile([128, NB], f32, tag="pso")
                nc.tensor.matmul(
                    pso[:], bds[p][:], qsmT[:, p, nbk * NB : nbk * NB + NB],
                    start=True, stop=True,
                )
                nc.scalar.copy(outT[:, p, :], pso[:])
            fo = p_fin.tile([128, NB // 128, DIM], f32, tag="fo")
            for lnt in range(NB // 128):
                psf = ps_fin.tile([128, DIM], f32, tag="psf")
                for ep in range(INNER // 128):
                    nc.tensor.matmul(
                        psf[:], outT[:, ep, lnt * 128 : lnt * 128 + 128],
                        wo_sb[:, ep, :],
                        start=(ep == 0), stop=(ep == INNER // 128 - 1),
                    )
                nc.vector.tensor_copy(fo[:, lnt, :], psf[:])
            nc.sync.dma_start(
                out.ap()[nbk * NB : (nbk + 1) * NB, :].rearrange(
                    "(i p) f -> p i f", p=128
                ),
                fo[:],
            )


def make_in_maps_v2(x, w_conv, wq, wk, wv, wo):
    x = np.asarray(x, np.float32)
    xTp = np.zeros((B, DIM, N + 2), np.float32)
    xTp[:, :, 1 : N + 1] = x.transpose(0, 2, 1)
    wct = np.ascontiguousarray(np.asarray(w_conv, np.float32).transpose(2, 1, 0))
    wq = np.ascontiguousarray(np.asarray(wq, np.float32))
    wk = np.ascontiguousarray(np.asarray(wk, np.float32))
    wv = np.ascontiguousarray(np.asarray(wv, np.float32))
    wo = np.ascontiguousarray(np.asarray(wo, np.float32))
    in_maps = []
    for c in range(8):
        b, s = c // 2, c % 2
        in_maps.append(
            {
                "xT": np.ascontiguousarray(xTp[b, :, s * NH : s * NH + NH + 2]),
                "wct": wct,
                "wq": wq,
                "wk": wk,
                "wv": wv,
                "wo": wo,
            }
        )
    return in_maps


def kernel_v2(x, w_conv, wq, wk, wv, wo, bo):
    key = ("v2", 1)
    if key not in _CACHE:
        _CACHE[key] = _Runner(build_kernel_v2(1), 8)
    runner = _CACHE[key]
    in_maps = make_in_maps_v2(x, w_conv, wq, wk, wv, wo)
    res = runner(in_maps)
    bo = np.asarray(bo, np.float32)
    out = np.empty((B, N, DIM), np.float32)
    for b in range(B):
        out[b, :NH] = res[2 * b]["out"]
        out[b, NH:] = res[2 * b + 1]["out"]
        out[b] += bo
    return out

